# revision 11
# baseline (speedup 1.0000x reference)
# Trainium2 Bass kernel for nn_LocalLayer (banded/local linear layer).
#
#   reference: y = x @ W.T + b
#     x [8192, 4096] f32, W [4096, 4096] f32 (block-banded: 256 windows x 16
#     outputs, window k reads inputs [16k-32, 16k+32) clipped to [0, 4096)),
#     b [4096] f32.
#
# Strategy (8 NeuronCores, data-parallel over batch):
#   - Host: transpose x -> xt [4096, 8192], shard batch 8 ways, zero-pad rows
#     by 32 (top) / 16 (bottom) -> per-core xt_pad [4144, 1024] in bf16.  The
#     -32 row shift makes every output tile's 176-wide input window a full
#     128-row chunk plus the first 48 rows of the next chunk.
#   - Host: gather W's band into compact stationary blocks:
#       w1[:, O*128+j][i] = W[128O+j, 128O-32+i]   (i in 0..127)
#       w2[:, O*128+j][i] = W[128O+j, 128O+96+i]   (i in 0..47)
#   - Precision: plain bf16 (gate is 2e-2; bf16 in / f32 psum / bf16 out
#     measures ~4e-3 max-rel-err).
#   - Device (per core): whole x shard and y shard live in SBUF (~130KB of
#     the 208KB partition budget), so DMA never waits on buffer recycling:
#     a few large multi-tile 3D DMAs stream x in on the Sync HWDGE ring
#     while matmuls chase them, and merged 4-tile output DMAs go out on the
#     Scalar HWDGE ring.  A dummy 12-matmul group on memset data warms the
#     PE clock (HAM un-throttle 1.2->2.4 GHz) during the DMA preamble.
#     Per output tile O (32) and batch chunk bc (2):
#       psum[128, 512] f32 = 2 accumulating bf16 matmuls (K=128 + K=48)
#       ys bf16 = psum + bias[:, O]  (ScalarE / VectorE alternating per O)
#   - Host: y = concat([yt_c.T for c in cores]).astype(f32).
#
# kernel() is self-contained: shapes/sharding hardcoded, no file reads.

import ml_dtypes
import numpy as np

import concourse.mybir as mybir
import concourse.tile as tile
from concourse import bacc
from concourse.bass_utils import run_bass_kernel_spmd

BF16 = ml_dtypes.bfloat16

BATCH = 8192
IN = 4096
N_CORES = 8
B_CORE = BATCH // N_CORES          # 1024
O_TILES = IN // 128                # 32
PAD_TOP = 32
K2 = 48                            # rows of the second (spill) chunk used
ROWS_PAD = O_TILES * 128 + K2      # 4144
BC = 512                           # batch chunk (one PSUM bank of f32)
N_BC = B_CORE // BC                # 2
X_GROUPS = [3, 4, 4, 4, 4, 4, 4, 4, 1]   # x-tile DMA batching (sum 32)
WARM_MM = 8

_NC_CACHE = {}


def _build_nc():
    if "nc" in _NC_CACHE:
        return _NC_CACHE["nc"]
    f32 = mybir.dt.float32
    bf16 = mybir.dt.bfloat16
    nc = bacc.Bacc("TRN2", target_bir_lowering=False, debug=False)
    xh_d = nc.dram_tensor("xh", [ROWS_PAD, B_CORE], bf16, kind="ExternalInput")
    w1_d = nc.dram_tensor("w1", [128, IN], bf16, kind="ExternalInput")
    w2_d = nc.dram_tensor("w2", [K2, IN], bf16, kind="ExternalInput")
    bias_d = nc.dram_tensor("bias", [128, O_TILES], f32, kind="ExternalInput")
    yt_d = nc.dram_tensor("yt", [IN, B_CORE], bf16, kind="ExternalOutput")

    def sb3(ap, tiles):   # SBUF [128, tiles*1024] view -> [128, tiles, 1024]
        return ap.rearrange("p (t c) -> p t c", t=tiles)

    def dr3(ap, tiles):   # DRAM [tiles*128, 1024] view -> [128, tiles, 1024]
        return ap.rearrange("(t p) c -> p t c", p=128)

    with tile.TileContext(nc) as tc:
        with (
            tc.tile_pool(name="consts", bufs=1) as cpool,
            tc.tile_pool(name="psum", bufs=7, space="PSUM") as ppool,
            tc.tile_pool(name="warm", bufs=1, space="PSUM") as wpool,
        ):
            w1_t = cpool.tile([128, IN], bf16, name="w1", tag="w1")
            w2_t = cpool.tile([K2, IN], bf16, name="w2", tag="w2")
            bias_t = cpool.tile([128, O_TILES], f32, name="bias")
            xs = cpool.tile([128, (O_TILES + 1) * B_CORE], bf16, name="xs")
            ys = cpool.tile([128, O_TILES * B_CORE], bf16, name="ys")
            wm = cpool.tile([128, 640], bf16, name="wm")

            # PE warm-up: dummy accumulation group on memset data, issued
            # before any DMA lands so HAM un-throttles during the preamble.
            nc.vector.memset(wm, 0.0)
            warm_ps = wpool.tile([128, BC], f32, tag="warm", name="warm_ps")
            for i in range(WARM_MM):
                nc.tensor.matmul(
                    warm_ps, wm[:, :128], wm[:, 128:640],
                    start=(i == 0), stop=(i == WARM_MM - 1),
                )

            # DMA issue order (Sync ring): bias first (tiny, gates every
            # activate via PSUM rotation), then first w chunk + first x
            # groups, then the rest interleaved so weights stay ahead.
            QW = IN // 4
            nc.sync.dma_start(bias_t, bias_d.ap())
            nc.sync.dma_start(w1_t[:, 0:QW], w1_d.ap()[:, 0:QW])
            nc.sync.dma_start(w2_t[:, 0:QW], w2_d.ap()[:, 0:QW])

            x_dmas = []
            r0 = 0
            for g, t in enumerate(X_GROUPS):
                sl_sb = sb3(xs[:, r0 * 8:(r0 + 128 * t) * 8], t)
                # (cols per row-tile: 1024 bf16; r0*8 == r0/128*1024)
                sl_dr = dr3(xh_d.ap()[r0:r0 + 128 * t, :], t)
                x_dmas.append((sl_sb, sl_dr))
                r0 += 128 * t
            assert r0 == O_TILES * 128
            # tail: first K2 rows of tile 32
            x_tail = (xs[:K2, O_TILES * B_CORE:(O_TILES + 1) * B_CORE],
                      xh_d.ap()[r0:r0 + K2, :])

            nc.sync.dma_start(*x_dmas[0])
            nc.sync.dma_start(*x_dmas[1])
            for q in range(1, 4):
                nc.sync.dma_start(w1_t[:, q * QW:(q + 1) * QW],
                                  w1_d.ap()[:, q * QW:(q + 1) * QW])
                nc.sync.dma_start(w2_t[:, q * QW:(q + 1) * QW],
                                  w2_d.ap()[:, q * QW:(q + 1) * QW])
                nc.sync.dma_start(*x_dmas[q + 1])
            for g in range(5, len(X_GROUPS)):
                nc.sync.dma_start(*x_dmas[g])
            nc.sync.dma_start(*x_tail)

            for O in range(O_TILES):
                osl = slice(O * 128, (O + 1) * 128)
                x0 = O * B_CORE
                x1 = (O + 1) * B_CORE
                pss = [
                    ppool.tile([128, BC], f32, tag="ps", name=f"ps_{O}_{i}")
                    for i in range(N_BC)
                ]
                # both w1 matmuls first: the w2 (spill) pair needs x tile
                # O+1, so this ordering buys ~1us of slack at group edges
                for bc in range(N_BC):
                    nc.tensor.matmul(
                        pss[bc], w1_t[:, osl], xs[:, x0 + bc * BC:x0 + (bc + 1) * BC],
                        start=True, stop=False,
                    )
                for bc in range(N_BC):
                    nc.tensor.matmul(
                        pss[bc], w2_t[:, osl], xs[:K2, x1 + bc * BC:x1 + (bc + 1) * BC],
                        start=False, stop=True,
                    )
                # filler matmuls: dependency-free PE work into a dedicated
                # PSUM bank, soaking up pipeline stalls so the HAM clock
                # gate never sees the PE idle (idle ~800ns => 1.2GHz).
                fps = wpool.tile([128, BC], f32, tag="warm", name=f"fps_{O}")
                nc.tensor.matmul(fps, wm[:K2, :128], wm[:K2, 128:640],
                                 start=True, stop=True)
                for bc in range(N_BC):
                    ysl = slice(x0 + bc * BC, x0 + (bc + 1) * BC)
                    if O % 2 == 0:
                        nc.scalar.add(ys[:, ysl], pss[bc], bias_t[:, O:O + 1])
                    else:
                        nc.vector.tensor_scalar_add(
                            ys[:, ysl], pss[bc], bias_t[:, O:O + 1]
                        )
                if O % 2 == 1:
                    g = O // 2
                    nc.sync.dma_start(
                        dr3(yt_d.ap()[g * 256:(g + 1) * 256, :], 2),
                        sb3(ys[:, g * 2 * B_CORE:(g + 1) * 2 * B_CORE], 2),
                    )

    nc.compile()
    _NC_CACHE["nc"] = nc
    return nc


def _band_gather(W, shift, rows):
    """wc[i, O*128+j] = W[128O+j, 128O+shift+i], zero outside [0, IN)."""
    i = np.arange(rows)[:, None, None]
    O = np.arange(O_TILES)[None, :, None]
    j = np.arange(128)[None, None, :]
    o_idx = np.broadcast_to(128 * O + j, (rows, O_TILES, 128))
    f = 128 * O + shift + i
    wc = np.where(
        (f >= 0) & (f < IN), W[o_idx, np.clip(f, 0, IN - 1)], np.float32(0)
    )
    return wc.reshape(rows, O_TILES * 128)


def kernel(x, W, b, mask=None):
    x = np.asarray(x, dtype=np.float32)
    W = np.asarray(W, dtype=np.float32)

    w1 = _band_gather(W, -PAD_TOP, 128).astype(BF16)
    w2 = _band_gather(W, 128 - PAD_TOP, K2).astype(BF16)
    bias = np.ascontiguousarray(
        np.asarray(b, dtype=np.float32).reshape(O_TILES, 128).T
    )

    xt = x.T  # [4096, 8192] view
    in_maps = []
    for c in range(N_CORES):
        sh = np.zeros((ROWS_PAD, B_CORE), BF16)
        sh[PAD_TOP:PAD_TOP + IN, :] = xt[:, c * B_CORE:(c + 1) * B_CORE]
        in_maps.append({"xh": sh, "w1": w1, "w2": w2, "bias": bias})

    nc = _build_nc()
    res = run_bass_kernel_spmd(nc, in_maps, core_ids=list(range(N_CORES)))
    y = np.concatenate(
        [np.asarray(r["yt"]).T.astype(np.float32) for r in res.results], axis=0
    )
    return np.ascontiguousarray(y)


if __name__ == "__main__":
    rng = np.random.default_rng(0)
    x = rng.standard_normal((BATCH, IN), dtype=np.float32)
    W = rng.standard_normal((IN, IN), dtype=np.float32)
    b = rng.standard_normal(IN, dtype=np.float32)
    y = kernel(x, W, b)
    print(y.shape, y.dtype)


# revision 12
# speedup vs baseline: 1.3876x; 1.3876x over previous
# Trainium2 Bass kernel for nn_LocalLayer (banded/local linear layer).
#
#   reference: y = x @ W.T + b
#     x [8192, 4096] f32, W [4096, 4096] f32 (block-banded: 256 windows x 16
#     outputs, window k reads inputs [16k-32, 16k+32) clipped to [0, 4096)),
#     b [4096] f32.
#
# Strategy (8 NeuronCores, data-parallel over batch):
#   - Host: transpose x -> xt [4096, 8192], shard batch 8 ways, zero-pad rows
#     by 32 (top) / 16 (bottom) -> per-core xt_pad [4144, 1024] in bf16.  The
#     -32 row shift makes every output tile's 176-wide input window a full
#     128-row chunk plus the first 48 rows of the next chunk.
#   - Host: gather W's band into compact stationary blocks:
#       w1[:, O*128+j][i] = W[128O+j, 128O-32+i]   (i in 0..127)
#       w2[:, O*128+j][i] = W[128O+j, 128O+96+i]   (i in 0..47)
#   - Precision: plain bf16 (gate is 2e-2; bf16 in / f32 psum / bf16 out
#     measures ~4e-3 max-rel-err).
#   - Device (per core): whole x shard and y shard live in SBUF (~130KB of
#     the 208KB partition budget), so DMA never waits on buffer recycling:
#     a few large multi-tile 3D DMAs stream x in on the Sync HWDGE ring
#     while matmuls chase them, and merged 4-tile output DMAs go out on the
#     Scalar HWDGE ring.  A dummy 12-matmul group on memset data warms the
#     PE clock (HAM un-throttle 1.2->2.4 GHz) during the DMA preamble.
#     Per output tile O (32) and batch chunk bc (2):
#       psum[128, 512] f32 = 2 accumulating bf16 matmuls (K=128 + K=48)
#       ys bf16 = psum + bias[:, O]  (ScalarE / VectorE alternating per O)
#   - Host: y = concat([yt_c.T for c in cores]).astype(f32).
#
# kernel() is self-contained: shapes/sharding hardcoded, no file reads.

import ml_dtypes
import numpy as np

import concourse.mybir as mybir
import concourse.tile as tile
from concourse import bacc
from concourse.bass_utils import run_bass_kernel_spmd

BF16 = ml_dtypes.bfloat16

BATCH = 8192
IN = 4096
N_CORES = 8
B_CORE = BATCH // N_CORES          # 1024
O_TILES = IN // 128                # 32
PAD_TOP = 32
K2 = 48                            # rows of the second (spill) chunk used
ROWS_PAD = O_TILES * 128 + K2      # 4144
BC = 512                           # batch chunk (one PSUM bank of f32)
N_BC = B_CORE // BC                # 2
X_GROUPS = [3, 4, 4, 4, 4, 4, 4, 4, 1]   # x-tile DMA batching (sum 32)
WARM_MM = 8

_NC_CACHE = {}


def _build_nc():
    if "nc" in _NC_CACHE:
        return _NC_CACHE["nc"]
    f32 = mybir.dt.float32
    bf16 = mybir.dt.bfloat16
    nc = bacc.Bacc("TRN2", target_bir_lowering=False, debug=False)
    xh_d = nc.dram_tensor("xh", [ROWS_PAD, B_CORE], bf16, kind="ExternalInput")
    w1_d = nc.dram_tensor("w1", [128, IN], bf16, kind="ExternalInput")
    w2_d = nc.dram_tensor("w2", [K2, IN], bf16, kind="ExternalInput")
    bias_d = nc.dram_tensor("bias", [128, O_TILES], f32, kind="ExternalInput")
    yt_d = nc.dram_tensor("yt", [IN, B_CORE], bf16, kind="ExternalOutput")

    def sb3(ap, tiles):   # SBUF [128, tiles*1024] view -> [128, tiles, 1024]
        return ap.rearrange("p (t c) -> p t c", t=tiles)

    def dr3(ap, tiles):   # DRAM [tiles*128, 1024] view -> [128, tiles, 1024]
        return ap.rearrange("(t p) c -> p t c", p=128)

    with tile.TileContext(nc) as tc:
        with (
            tc.tile_pool(name="consts", bufs=1) as cpool,
            tc.tile_pool(name="psum", bufs=7, space="PSUM") as ppool,
            tc.tile_pool(name="warm", bufs=1, space="PSUM") as wpool,
        ):
            w1_t = cpool.tile([128, IN], bf16, name="w1", tag="w1")
            w2_t = cpool.tile([K2, IN], bf16, name="w2", tag="w2")
            bias_t = cpool.tile([128, O_TILES], f32, name="bias")
            xs = cpool.tile([128, (O_TILES + 1) * B_CORE], bf16, name="xs")
            ys = cpool.tile([128, O_TILES * B_CORE], bf16, name="ys")
            wm = cpool.tile([128, 640], bf16, name="wm")

            # PE warm-up: dummy accumulation group on memset data, issued
            # before any DMA lands so HAM un-throttles during the preamble.
            nc.vector.memset(wm, 0.0)
            warm_ps = wpool.tile([128, BC], f32, tag="warm", name="warm_ps")
            for i in range(WARM_MM):
                nc.tensor.matmul(
                    warm_ps, wm[:, :128], wm[:, 128:640],
                    start=(i == 0), stop=(i == WARM_MM - 1),
                )

            # DMA issue order (Sync ring): bias first (tiny, gates every
            # activate via PSUM rotation), then first w chunk + first x
            # groups, then the rest interleaved so weights stay ahead.
            QW = IN // 4
            nc.sync.dma_start(bias_t, bias_d.ap())
            nc.sync.dma_start(w1_t[:, 0:QW], w1_d.ap()[:, 0:QW])
            nc.sync.dma_start(w2_t[:, 0:QW], w2_d.ap()[:, 0:QW])

            x_dmas = []
            r0 = 0
            for g, t in enumerate(X_GROUPS):
                sl_sb = sb3(xs[:, r0 * 8:(r0 + 128 * t) * 8], t)
                # (cols per row-tile: 1024 bf16; r0*8 == r0/128*1024)
                sl_dr = dr3(xh_d.ap()[r0:r0 + 128 * t, :], t)
                x_dmas.append((sl_sb, sl_dr))
                r0 += 128 * t
            assert r0 == O_TILES * 128
            # tail: first K2 rows of tile 32
            x_tail = (xs[:K2, O_TILES * B_CORE:(O_TILES + 1) * B_CORE],
                      xh_d.ap()[r0:r0 + K2, :])

            nc.sync.dma_start(*x_dmas[0])
            nc.sync.dma_start(*x_dmas[1])
            for q in range(1, 4):
                nc.sync.dma_start(w1_t[:, q * QW:(q + 1) * QW],
                                  w1_d.ap()[:, q * QW:(q + 1) * QW])
                nc.sync.dma_start(w2_t[:, q * QW:(q + 1) * QW],
                                  w2_d.ap()[:, q * QW:(q + 1) * QW])
                nc.sync.dma_start(*x_dmas[q + 1])
            for g in range(5, len(X_GROUPS)):
                nc.sync.dma_start(*x_dmas[g])
            nc.sync.dma_start(*x_tail)

            for O in range(O_TILES):
                osl = slice(O * 128, (O + 1) * 128)
                x0 = O * B_CORE
                x1 = (O + 1) * B_CORE
                pss = [
                    ppool.tile([128, BC], f32, tag="ps", name=f"ps_{O}_{i}")
                    for i in range(N_BC)
                ]
                # both w1 matmuls first: the w2 (spill) pair needs x tile
                # O+1, so this ordering buys ~1us of slack at group edges
                for bc in range(N_BC):
                    nc.tensor.matmul(
                        pss[bc], w1_t[:, osl], xs[:, x0 + bc * BC:x0 + (bc + 1) * BC],
                        start=True, stop=False,
                    )
                for bc in range(N_BC):
                    nc.tensor.matmul(
                        pss[bc], w2_t[:, osl], xs[:K2, x1 + bc * BC:x1 + (bc + 1) * BC],
                        start=False, stop=True,
                    )
                # filler matmuls: dependency-free PE work into a dedicated
                # PSUM bank, soaking up pipeline stalls so the HAM clock
                # gate never sees the PE idle (idle ~800ns => 1.2GHz).
                fps = wpool.tile([128, BC], f32, tag="warm", name=f"fps_{O}")
                for _ in range(2):
                    nc.tensor.matmul(fps, wm[:, :128], wm[:, 128:640],
                                     start=True, stop=True)
                for bc in range(N_BC):
                    ysl = slice(x0 + bc * BC, x0 + (bc + 1) * BC)
                    if O % 2 == 0:
                        nc.scalar.add(ys[:, ysl], pss[bc], bias_t[:, O:O + 1])
                    else:
                        nc.vector.tensor_scalar_add(
                            ys[:, ysl], pss[bc], bias_t[:, O:O + 1]
                        )
                if O % 2 == 1:
                    g = O // 2
                    nc.sync.dma_start(
                        dr3(yt_d.ap()[g * 256:(g + 1) * 256, :], 2),
                        sb3(ys[:, g * 2 * B_CORE:(g + 1) * 2 * B_CORE], 2),
                    )

    nc.compile()
    _NC_CACHE["nc"] = nc
    return nc


def _band_gather(W, shift, rows):
    """wc[i, O*128+j] = W[128O+j, 128O+shift+i], zero outside [0, IN)."""
    i = np.arange(rows)[:, None, None]
    O = np.arange(O_TILES)[None, :, None]
    j = np.arange(128)[None, None, :]
    o_idx = np.broadcast_to(128 * O + j, (rows, O_TILES, 128))
    f = 128 * O + shift + i
    wc = np.where(
        (f >= 0) & (f < IN), W[o_idx, np.clip(f, 0, IN - 1)], np.float32(0)
    )
    return wc.reshape(rows, O_TILES * 128)


def kernel(x, W, b, mask=None):
    x = np.asarray(x, dtype=np.float32)
    W = np.asarray(W, dtype=np.float32)

    w1 = _band_gather(W, -PAD_TOP, 128).astype(BF16)
    w2 = _band_gather(W, 128 - PAD_TOP, K2).astype(BF16)
    bias = np.ascontiguousarray(
        np.asarray(b, dtype=np.float32).reshape(O_TILES, 128).T
    )

    xt = x.T  # [4096, 8192] view
    in_maps = []
    for c in range(N_CORES):
        sh = np.zeros((ROWS_PAD, B_CORE), BF16)
        sh[PAD_TOP:PAD_TOP + IN, :] = xt[:, c * B_CORE:(c + 1) * B_CORE]
        in_maps.append({"xh": sh, "w1": w1, "w2": w2, "bias": bias})

    nc = _build_nc()
    res = run_bass_kernel_spmd(nc, in_maps, core_ids=list(range(N_CORES)))
    y = np.concatenate(
        [np.asarray(r["yt"]).T.astype(np.float32) for r in res.results], axis=0
    )
    return np.ascontiguousarray(y)


if __name__ == "__main__":
    rng = np.random.default_rng(0)
    x = rng.standard_normal((BATCH, IN), dtype=np.float32)
    W = rng.standard_normal((IN, IN), dtype=np.float32)
    b = rng.standard_normal(IN, dtype=np.float32)
    y = kernel(x, W, b)
    print(y.shape, y.dtype)


# revision 13
# speedup vs baseline: 1.5369x; 1.1076x over previous
# Trainium2 Bass kernel for nn_LocalLayer (banded/local linear layer).
#
#   reference: y = x @ W.T + b
#     x [8192, 4096] f32, W [4096, 4096] f32 (block-banded: 256 windows x 16
#     outputs, window k reads inputs [16k-32, 16k+32) clipped to [0, 4096)),
#     b [4096] f32.
#
# Strategy (8 NeuronCores, data-parallel over batch):
#   - Host: transpose x -> xt [4096, 8192], shard batch 8 ways, zero-pad rows
#     by 32 (top) / 16 (bottom) -> per-core xt_pad [4144, 1024] in bf16.  The
#     -32 row shift makes every output tile's 176-wide input window a full
#     128-row chunk plus the first 48 rows of the next chunk.
#   - Host: gather W's band into compact stationary blocks:
#       w1[:, O*128+j][i] = W[128O+j, 128O-32+i]   (i in 0..127)
#       w2[:, O*128+j][i] = W[128O+j, 128O+96+i]   (i in 0..47)
#   - Precision: plain bf16 (gate is 2e-2; bf16 in / f32 psum / bf16 out
#     measures ~4e-3 max-rel-err).
#   - Device (per core): whole x shard and y shard live in SBUF (~130KB of
#     the 208KB partition budget), so DMA never waits on buffer recycling:
#     a few large multi-tile 3D DMAs stream x in on the Sync HWDGE ring
#     while matmuls chase them, and merged 4-tile output DMAs go out on the
#     Scalar HWDGE ring.  A dummy 12-matmul group on memset data warms the
#     PE clock (HAM un-throttle 1.2->2.4 GHz) during the DMA preamble.
#     Per output tile O (32) and batch chunk bc (2):
#       psum[128, 512] f32 = 2 accumulating bf16 matmuls (K=128 + K=48)
#       ys bf16 = psum + bias[:, O]  (ScalarE / VectorE alternating per O)
#   - Host: y = concat([yt_c.T for c in cores]).astype(f32).
#
# kernel() is self-contained: shapes/sharding hardcoded, no file reads.

import ml_dtypes
import numpy as np

import concourse.mybir as mybir
import concourse.tile as tile
from concourse import bacc
from concourse.bass_utils import run_bass_kernel_spmd

BF16 = ml_dtypes.bfloat16

BATCH = 8192
IN = 4096
N_CORES = 8
B_CORE = BATCH // N_CORES          # 1024
O_TILES = IN // 128                # 32
PAD_TOP = 32
ROWS_PAD = (O_TILES + 1) * 128     # 4224 (32 zeros top, 96 zeros bottom)
BC = 512                           # batch chunk (one PSUM bank of f32)
N_BC = B_CORE // BC                # 2
X_GROUPS = [3, 4, 4, 4, 4, 4, 4, 4, 2]   # x-tile DMA batching (sum 33)
WARM_MM = 8

_NC_CACHE = {}


def _build_nc():
    if "nc" in _NC_CACHE:
        return _NC_CACHE["nc"]
    f32 = mybir.dt.float32
    bf16 = mybir.dt.bfloat16
    nc = bacc.Bacc("TRN2", target_bir_lowering=False, debug=False)
    xh_d = nc.dram_tensor("xh", [ROWS_PAD, B_CORE], bf16, kind="ExternalInput")
    w1_d = nc.dram_tensor("w1", [128, IN], bf16, kind="ExternalInput")
    w2_d = nc.dram_tensor("w2", [128, IN], bf16, kind="ExternalInput")
    bias_d = nc.dram_tensor("bias", [128, O_TILES], f32, kind="ExternalInput")
    yt_d = nc.dram_tensor("yt", [IN, B_CORE], bf16, kind="ExternalOutput")

    def sb3(ap, tiles):   # SBUF [128, tiles*1024] view -> [128, tiles, 1024]
        return ap.rearrange("p (t c) -> p t c", t=tiles)

    def dr3(ap, tiles):   # DRAM [tiles*128, 1024] view -> [128, tiles, 1024]
        return ap.rearrange("(t p) c -> p t c", p=128)

    with tile.TileContext(nc) as tc:
        with (
            tc.tile_pool(name="consts", bufs=1) as cpool,
            tc.tile_pool(name="psum", bufs=8, space="PSUM") as ppool,
        ):
            w1_t = cpool.tile([128, IN], bf16, name="w1", tag="w1")
            w2_t = cpool.tile([128, IN], bf16, name="w2", tag="w2")
            bias_t = cpool.tile([128, O_TILES], f32, name="bias")
            xs = cpool.tile([128, (O_TILES + 1) * B_CORE], bf16, name="xs")
            ys = cpool.tile([128, O_TILES * B_CORE], bf16, name="ys")
            wm = cpool.tile([128, 640], bf16, name="wm")

            # PE warm-up: dummy accumulation group on memset data, issued
            # before any DMA lands so HAM un-throttles during the preamble.
            nc.vector.memset(wm, 0.0)
            warm_ps = ppool.tile([128, BC], f32, tag="ps", name="warm_ps")
            for i in range(WARM_MM):
                nc.tensor.matmul(
                    warm_ps, wm[:, :128], wm[:, 128:640],
                    start=(i == 0), stop=(i == WARM_MM - 1),
                )

            # DMA issue order (Sync ring): bias first (tiny, gates every
            # activate via PSUM rotation), then first w chunk + first x
            # groups, then the rest interleaved so weights stay ahead.
            QW = IN // 4
            nc.sync.dma_start(bias_t, bias_d.ap())
            nc.sync.dma_start(w1_t[:, 0:QW], w1_d.ap()[:, 0:QW])
            nc.sync.dma_start(w2_t[:, 0:QW], w2_d.ap()[:, 0:QW])

            x_dmas = []
            r0 = 0
            for g, t in enumerate(X_GROUPS):
                sl_sb = sb3(xs[:, r0 * 8:(r0 + 128 * t) * 8], t)
                # (cols per row-tile: 1024 bf16; r0*8 == r0/128*1024)
                sl_dr = dr3(xh_d.ap()[r0:r0 + 128 * t, :], t)
                x_dmas.append((sl_sb, sl_dr))
                r0 += 128 * t
            assert r0 == (O_TILES + 1) * 128

            nc.sync.dma_start(*x_dmas[0])
            nc.sync.dma_start(*x_dmas[1])
            for q in range(1, 4):
                nc.sync.dma_start(w1_t[:, q * QW:(q + 1) * QW],
                                  w1_d.ap()[:, q * QW:(q + 1) * QW])
                nc.sync.dma_start(w2_t[:, q * QW:(q + 1) * QW],
                                  w2_d.ap()[:, q * QW:(q + 1) * QW])
                nc.sync.dma_start(*x_dmas[q + 1])
            for g in range(5, len(X_GROUPS)):
                nc.sync.dma_start(*x_dmas[g])

            for O in range(O_TILES):
                osl = slice(O * 128, (O + 1) * 128)
                x0 = O * B_CORE
                x1 = (O + 1) * B_CORE
                pss = [
                    ppool.tile([128, BC], f32, tag="ps", name=f"ps_{O}_{i}")
                    for i in range(N_BC)
                ]
                # both w1 matmuls first: the w2 (spill) pair needs x tile
                # O+1, so this ordering buys ~1us of slack at group edges
                for bc in range(N_BC):
                    nc.tensor.matmul(
                        pss[bc], w1_t[:, osl], xs[:, x0 + bc * BC:x0 + (bc + 1) * BC],
                        start=True, stop=False,
                    )
                for bc in range(N_BC):
                    nc.tensor.matmul(
                        pss[bc], w2_t[:, osl], xs[:, x1 + bc * BC:x1 + (bc + 1) * BC],
                        start=False, stop=True,
                    )
                for bc in range(N_BC):
                    ysl = slice(x0 + bc * BC, x0 + (bc + 1) * BC)
                    if O % 2 == 0:
                        nc.scalar.add(ys[:, ysl], pss[bc], bias_t[:, O:O + 1])
                    else:
                        nc.vector.tensor_scalar_add(
                            ys[:, ysl], pss[bc], bias_t[:, O:O + 1]
                        )
                if O % 2 == 1:
                    g = O // 2
                    nc.sync.dma_start(
                        dr3(yt_d.ap()[g * 256:(g + 1) * 256, :], 2),
                        sb3(ys[:, g * 2 * B_CORE:(g + 1) * 2 * B_CORE], 2),
                    )

    nc.compile()
    _NC_CACHE["nc"] = nc
    return nc


def _band_gather(W, shift, rows):
    """wc[i, O*128+j] = W[128O+j, 128O+shift+i], zero outside [0, IN)."""
    i = np.arange(rows)[:, None, None]
    O = np.arange(O_TILES)[None, :, None]
    j = np.arange(128)[None, None, :]
    o_idx = np.broadcast_to(128 * O + j, (rows, O_TILES, 128))
    f = 128 * O + shift + i
    wc = np.where(
        (f >= 0) & (f < IN), W[o_idx, np.clip(f, 0, IN - 1)], np.float32(0)
    )
    return wc.reshape(rows, O_TILES * 128)


def kernel(x, W, b, mask=None):
    x = np.asarray(x, dtype=np.float32)
    W = np.asarray(W, dtype=np.float32)

    w1 = _band_gather(W, -PAD_TOP, 128).astype(BF16)
    w2 = _band_gather(W, 128 - PAD_TOP, 128).astype(BF16)
    bias = np.ascontiguousarray(
        np.asarray(b, dtype=np.float32).reshape(O_TILES, 128).T
    )

    xt = x.T  # [4096, 8192] view
    in_maps = []
    for c in range(N_CORES):
        sh = np.zeros((ROWS_PAD, B_CORE), BF16)
        sh[PAD_TOP:PAD_TOP + IN, :] = xt[:, c * B_CORE:(c + 1) * B_CORE]
        in_maps.append({"xh": sh, "w1": w1, "w2": w2, "bias": bias})

    nc = _build_nc()
    res = run_bass_kernel_spmd(nc, in_maps, core_ids=list(range(N_CORES)))
    y = np.concatenate(
        [np.asarray(r["yt"]).T.astype(np.float32) for r in res.results], axis=0
    )
    return np.ascontiguousarray(y)


if __name__ == "__main__":
    rng = np.random.default_rng(0)
    x = rng.standard_normal((BATCH, IN), dtype=np.float32)
    W = rng.standard_normal((IN, IN), dtype=np.float32)
    b = rng.standard_normal(IN, dtype=np.float32)
    y = kernel(x, W, b)
    print(y.shape, y.dtype)
